# revision 2
# baseline (speedup 1.0000x reference)
"""ChildSum TreeLSTM cell on 8 Trainium2 NeuronCores — V3.

Data-parallel over nodes (N/8 per core). Feature dims on SBUF partitions,
nodes on the free dim, 1024-node tiles.

Key structure per tile (vs the bf16 baseline):
  - Gate pre-acts accumulate in ONE [128,2,1024] PSUM (j-pair, scale 64):
    wx via 3 fp16 matmuls per (j,half) with the bias folded into x's
    constant-1 padding row, then per-k fp8 e4m3 DoubleRow matmuls
    (256-contraction) swap uh_{k-1} out / uh_k in exactly (+-U64), so the
    four forget gates share one PSUM and one sigmoid batch (FD=2048).
  - iou stays fp16 (precision-critical tanh path), biases matmul-folded.
  - All elementwise in fp16 as j-pair FD=2048 DVE ops.
  - h_msgs ship as e4m3 (half DMA), c/x/h_tild/out as fp16.
"""

import os

os.environ.setdefault("JAX_COMPILATION_CACHE_DIR", "/root/.cache/jax_bass")

import numpy as np
import ml_dtypes

import concourse.bass as bass
import concourse.mybir as mybir
import concourse.tile as tile
from concourse import bacc
from concourse.bass_utils import run_bass_kernel_spmd

E4 = ml_dtypes.float8_e4m3
F16 = np.float16
F32 = np.float32

N_CORES = 8
N_FULL = 65536
NSH = N_FULL // N_CORES
H = 256
X_SIZE = 300
XP = 384  # x padded to 3*128 (row 300 = 1.0 carries the biases)
K = 4
TN = 1024

SIG = mybir.ActivationFunctionType.Sigmoid
TANH = mybir.ActivationFunctionType.Tanh
DR = mybir.MatmulPerfMode.DoubleRow

LAST_RESULTS = None


def build_bass(nsh=NSH, tn=TN):
    f32 = mybir.dt.float32
    f16 = mybir.dt.float16
    f8 = mybir.dt.float8e4
    nt = nsh // tn
    assert nsh % tn == 0

    nc = bacc.Bacc("TRN2", debug=False)

    xt = nc.dram_tensor("xt", [3, 128, nsh], f16, kind="ExternalInput")
    h8 = nc.dram_tensor("h8", [K, 2, 128, nsh], f8, kind="ExternalInput")
    c16 = nc.dram_tensor("c16", [K, 2, 128, nsh], f16, kind="ExternalInput")
    ht16 = nc.dram_tensor("ht16", [2, 128, nsh], f16, kind="ExternalInput")
    wf = nc.dram_tensor("wf", [3, 128, 2, 128], f16, kind="ExternalInput")
    uf = nc.dram_tensor("uf", [6, 128, 2, 128], f8, kind="ExternalInput")
    wiou = nc.dram_tensor("wiou", [3, 128, 6, 128], f16, kind="ExternalInput")
    uiou = nc.dram_tensor("uiou", [2, 128, 6, 128], f16, kind="ExternalInput")
    out = nc.dram_tensor("out", [2, 2, 128, nsh], f16, kind="ExternalOutput")

    nh = tn // 512  # psum halves per 1024-node chunk

    with tile.TileContext(nc) as tc:
        with (
            tc.tile_pool(name="consts", bufs=1) as consts,
            tc.tile_pool(name="xin", bufs=2) as xin,
            tc.tile_pool(name="hin", bufs=2) as hin,
            tc.tile_pool(name="cin", bufs=2) as cin,
            tc.tile_pool(name="htin", bufs=2) as htin,
            tc.tile_pool(name="fpool", bufs=2) as fpool,
            tc.tile_pool(name="ppool", bufs=2) as ppool,
            tc.tile_pool(name="accp", bufs=3) as accp,
            tc.tile_pool(name="ioup", bufs=2) as ioup,
            tc.tile_pool(name="outp", bufs=2) as outp,
            tc.tile_pool(name="gps", bufs=1, space="PSUM") as gpsp,
            tc.tile_pool(name="iops", bufs=2, space="PSUM") as iopsp,
        ):
            wf_s = consts.tile([128, 3, 2, 128], f16)
            nc.sync.dma_start(wf_s[:], wf[:].rearrange("k p j o -> p k j o"))
            uf_s = consts.tile([128, 6, 2, 128], f8)
            nc.sync.dma_start(uf_s[:], uf[:].rearrange("s p j o -> p s j o"))
            wiou_s = consts.tile([128, 3, 6, 128], f16)
            nc.sync.dma_start(wiou_s[:], wiou[:].rearrange("k p c o -> p k c o"))
            uiou_s = consts.tile([128, 2, 6, 128], f16)
            nc.sync.dma_start(uiou_s[:], uiou[:].rearrange("k p c o -> p k c o"))

            for t in range(nt):
                n0 = t * tn
                nsl = slice(n0, n0 + tn)

                xtile = xin.tile([128, 3, tn], f16, tag="x")
                nc.sync.dma_start(xtile[:], xt[:, :, nsl].rearrange("k p n -> p k n"))
                htile = hin.tile([128, K, 2, tn], f8, tag="h")
                nc.sync.dma_start(
                    htile[:], h8[:, :, :, nsl].rearrange("k c p n -> p k c n")
                )
                ctile = cin.tile([128, K, 2, tn], f16, tag="c")
                nc.sync.dma_start(
                    ctile[:], c16[:, :, :, nsl].rearrange("k c p n -> p k c n")
                )
                httile = htin.tile([128, 2, tn], f16, tag="ht")
                nc.sync.dma_start(
                    httile[:], ht16[:, :, nsl].rearrange("c p n -> p c n")
                )

                gps = gpsp.tile([128, 2, tn], f32, tag="g")
                gi = ioup.tile([128, 2, tn], f16, tag="i")
                go = ioup.tile([128, 2, tn], f16, tag="o")
                gu = ioup.tile([128, 2, tn], f16, tag="u")
                gp = {"i": gi, "o": go, "u": gu}

                def iou_chunk(g, j):
                    """one iou output chunk: fp16 matmuls + inline activation."""
                    cidx = 2 * g + j
                    ps = iopsp.tile([128, tn], f32, tag="io")
                    for s in range(nh):
                        ssl = slice(s * 512, (s + 1) * 512)
                        psl = ps[:, ssl]
                        for kt in range(3):
                            nc.tensor.matmul(
                                psl, wiou_s[:, kt, cidx, :], xtile[:, kt, ssl],
                                start=(kt == 0), stop=False,
                            )
                        for kt in range(2):
                            nc.tensor.matmul(
                                psl, uiou_s[:, kt, cidx, :], httile[:, kt, ssl],
                                start=False, stop=(kt == 1),
                            )
                    name, func = (("i", SIG), ("o", SIG), ("u", TANH))[g]
                    nc.scalar.activation(gp[name][:, j, :], ps[:], func)

                # ---- gate block: k0 then swap k1..k3, sigma after each ----
                # iou chunks are interleaved between swap steps to keep the PE
                # busy while ACT drains the gate psum.
                for j in range(2):
                    for s in range(nh):
                        ssl = slice(s * 512, (s + 1) * 512)
                        psl = gps[:, j, ssl]
                        for kt in range(3):
                            nc.tensor.matmul(
                                psl, wf_s[:, kt, j, :], xtile[:, kt, ssl],
                                start=(kt == 0), stop=False,
                            )
                        # k0: +U64 both h-chunks in one DoubleRow
                        nc.tensor.matmul(
                            psl, uf_s[:, 0:2, j, :], htile[:, 0, :, ssl],
                            start=False, stop=False, perf_mode=DR,
                        )

                ftiles = []
                fk = fpool.tile([128, 2, tn], f16, tag="f0")
                nc.scalar.activation(fk[:], gps[:], SIG, scale=1.0 / 64.0)
                ftiles.append(fk)

                iou_order = [(0, 0), (0, 1), (1, 0), (1, 1), (2, 0), (2, 1)]
                io_i = 0

                for k in range(1, K):
                    # interleave one iou chunk before each swap round
                    g, j = iou_order[io_i]
                    iou_chunk(g, j)
                    io_i += 1

                    last = k == K - 1
                    for j in range(2):
                        for s in range(nh):
                            ssl = slice(s * 512, (s + 1) * 512)
                            psl = gps[:, j, ssl]
                            # swap: (h_{k-1}c, -U64c) + (h_k c, +U64c) per chunk c
                            for c in range(2):
                                nc.tensor.matmul(
                                    psl,
                                    uf_s[:, 2 + 2 * c:2 + 2 * c + 2, j, :],
                                    htile[:, k - 1:k + 1, c, ssl],
                                    start=False,
                                    stop=(last and c == 1),
                                    perf_mode=DR,
                                )
                    fk = fpool.tile([128, 2, tn], f16, tag=f"f{k}")
                    nc.scalar.activation(fk[:], gps[:], SIG, scale=1.0 / 64.0)
                    ftiles.append(fk)

                # remaining iou chunks
                while io_i < 6:
                    g, j = iou_order[io_i]
                    iou_chunk(g, j)
                    io_i += 1

                # ---- c_tild tree (j-pair FD=2048 f16 ops) ----
                sums = []
                for k2 in range(2):
                    pa = ppool.tile([128, 2, tn], f16, tag="pa")
                    nc.vector.tensor_mul(
                        pa[:], ftiles[2 * k2][:], ctile[:, 2 * k2, :, :]
                    )
                    pb = ppool.tile([128, 2, tn], f16, tag="pb")
                    nc.vector.tensor_mul(
                        pb[:], ftiles[2 * k2 + 1][:], ctile[:, 2 * k2 + 1, :, :]
                    )
                    sk = accp.tile([128, 2, tn], f16, tag="s")
                    nc.vector.tensor_add(sk[:], pa[:], pb[:])
                    sums.append(sk)
                ctild = accp.tile([128, 2, tn], f16, tag="ctild")
                nc.vector.tensor_add(ctild[:], sums[0][:], sums[1][:])

                # ---- outputs ----
                ciu = outp.tile([128, 2, tn], f16, tag="ciu")
                nc.vector.tensor_mul(ciu[:], gp["i"][:], gp["u"][:])
                cout = outp.tile([128, 2, tn], f16, tag="cout")
                nc.vector.tensor_add(cout[:], ciu[:], ctild[:])
                th = outp.tile([128, 2, tn], f16, tag="th")
                nc.scalar.activation(th[:], cout[:], TANH)
                hout = outp.tile([128, 2, tn], f16, tag="hout")
                nc.vector.tensor_mul(hout[:], gp["o"][:], th[:])

                nc.sync.dma_start(
                    out[0, :, :, nsl].rearrange("j p n -> p j n"), hout[:]
                )
                nc.sync.dma_start(
                    out[1, :, :, nsl].rearrange("j p n -> p j n"), cout[:]
                )

    nc.compile()
    return nc


_NC_CACHE = {}


def _get_nc(nsh, tn):
    key = (nsh, tn)
    if key not in _NC_CACHE:
        _NC_CACHE[key] = build_bass(nsh, tn)
    return _NC_CACHE[key]


def prep_host_inputs(x, h_msgs, c_msgs, W_iou, b_iou, U_iou, b_Uiou, W_f, b_Wf, U_f, b_Uf):
    n = x.shape[0]
    nsh = n // N_CORES
    x = np.asarray(x, F32)
    h_msgs = np.asarray(h_msgs, F32)
    c_msgs = np.asarray(c_msgs, F32)

    # --- moving streams ---
    xp = np.zeros((XP, n), F32)
    xp[:X_SIZE] = x.T
    xp[X_SIZE] = 1.0  # bias carrier row
    xt_full = xp.astype(F16).reshape(3, 128, n)

    h8_full = np.ascontiguousarray(h_msgs.astype(E4).transpose(1, 2, 0)).reshape(
        K, 2, 128, n
    )
    c16_full = np.ascontiguousarray(c_msgs.astype(F16).transpose(1, 2, 0)).reshape(
        K, 2, 128, n
    )
    ht = h_msgs.sum(1)  # [n, H] fp32
    ht16_full = np.ascontiguousarray(ht.astype(F16).T).reshape(2, 128, n)

    # --- stationary weights ---
    # gate x-part at scale 64, bias folded on row 300
    wfp = np.zeros((XP, H), F32)
    wfp[:X_SIZE] = 64.0 * np.asarray(W_f, F32).T
    wfp[X_SIZE] = 64.0 * (np.asarray(b_Wf, F32) + np.asarray(b_Uf, F32))
    wf_host = wfp.astype(F16).reshape(3, 128, 2, 128)

    # uh at scale 64 in e4m3; slots [+c0, +c1, -c0, +c0, -c1, +c1]
    u64 = (64.0 * np.asarray(U_f, F32).T).astype(E4).astype(F32)  # [H_in, H_out]
    u64_k = u64.reshape(2, 128, 2, 128)  # [c, p, j, o]
    uf_host = np.zeros((6, 128, 2, 128), F32)
    uf_host[0] = u64_k[0]
    uf_host[1] = u64_k[1]
    uf_host[2] = -u64_k[0]
    uf_host[3] = u64_k[0]
    uf_host[4] = -u64_k[1]
    uf_host[5] = u64_k[1]
    uf_host = uf_host.astype(E4)

    # iou fp16, bias folded; output chunk order c = 2*gate + j
    wioup = np.zeros((XP, 3 * H), F32)
    wioup[:X_SIZE] = np.asarray(W_iou, F32).T
    wioup[X_SIZE] = np.asarray(b_iou, F32) + np.asarray(b_Uiou, F32)
    wiou_host = wioup.astype(F16).reshape(3, 128, 6, 128)
    uiou_host = np.ascontiguousarray(np.asarray(U_iou, F32).T).astype(F16).reshape(
        2, 128, 6, 128
    )

    in_maps = []
    for cix in range(N_CORES):
        sl = slice(cix * nsh, (cix + 1) * nsh)
        in_maps.append(
            {
                "xt": np.ascontiguousarray(xt_full[:, :, sl]),
                "h8": np.ascontiguousarray(h8_full[:, :, :, sl]),
                "c16": np.ascontiguousarray(c16_full[:, :, :, sl]),
                "ht16": np.ascontiguousarray(ht16_full[:, :, sl]),
                "wf": wf_host,
                "uf": uf_host,
                "wiou": wiou_host,
                "uiou": uiou_host,
            }
        )
    return in_maps


def kernel(**inputs):
    global LAST_RESULTS
    inputs = {k: np.asarray(v) for k, v in inputs.items()}
    n = inputs["x"].shape[0]
    assert n == N_FULL, f"hardcoded for N={N_FULL}, got {n}"
    nsh = n // N_CORES

    nc = _get_nc(nsh, TN)
    in_maps = prep_host_inputs(**inputs)

    res = None
    for attempt in range(3):
        try:
            res = run_bass_kernel_spmd(nc, in_maps, core_ids=list(range(N_CORES)))
            break
        except Exception:
            if attempt == 2:
                raise
            import time as _time

            _time.sleep(5.0)
    LAST_RESULTS = res

    # results[c]["out"]: [2, 2, 128, nsh] -> full [2, N, 256]
    per_core = [r["out"].astype(F32).reshape(2, 256, nsh) for r in res.results]
    full = np.concatenate(per_core, axis=-1)
    return np.ascontiguousarray(full.transpose(0, 2, 1)).astype(F32)
